# revision 6
# baseline (speedup 1.0000x reference)
"""Mega (Flash-Quad) encoder layer on 8 Trainium2 NeuronCores.

Sharding: data-parallel over batch B=16 -> Bc=2 per core. Per core:
MultiHeadEMA as 4 first-order DVE scans per 128-channel block (2 EMA dims x 2
directions; decay/weight scalars precomputed on host), fp16 PE matmuls with
host-pretransposed x^T, feature-major z/r so the attention AV matmul needs no
on-chip transposes, chunked softmax attention, gated residual + ScaleNorm per
128-token chunk.
"""

import numpy as np

L, B, D = 2048, 16, 512
H, Z, NDIM = 1024, 128, 2
CHUNK = 128
MAXPOS = 1024
EPS = 1e-6
N_CORES = 8
BC = B // N_CORES          # batches per core
NCH = L // CHUNK           # chunks per batch
DO = D // 128              # d_outer tiles
HO = H // 128              # h_outer tiles

_PROG_CACHE = {}


def _build_program(zero_bias: bool):
    key = ("nc", zero_bias)
    if key in _PROG_CACHE:
        return _PROG_CACHE[key]

    import concourse.mybir as mybir
    import concourse.tile as tile
    from concourse import bacc
    from concourse.masks import make_identity

    f32 = mybir.dt.float32
    f16 = mybir.dt.float16
    AF = mybir.ActivationFunctionType
    OP = mybir.AluOpType
    AX = mybir.AxisListType

    nc = bacc.Bacc(target_bir_lowering=False)

    x_in = nc.declare_dram_parameter("x", [L, BC, D], f32, isOutput=False)
    xT_in = nc.declare_dram_parameter("xT16", [BC, DO, 128, L], f16, isOutput=False)
    Wv_in = nc.declare_dram_parameter("Wv16", [D, H], f16, isOutput=False)
    Wmx_in = nc.declare_dram_parameter("Wmx16", [D, 2 * D + H + Z], f16, isOutput=False)
    Wh_in = nc.declare_dram_parameter("Wh16", [H, D], f16, isOutput=False)
    emaq_in = nc.declare_dram_parameter("ema_q", [DO, 128, 4], f32, isOutput=False)
    emaw_in = nc.declare_dram_parameter("ema_w", [DO, 128, 4], f32, isOutput=False)
    omega_in = nc.declare_dram_parameter("omega", [DO, 128, 1], f32, isOutput=False)
    qkaff_in = nc.declare_dram_parameter("qk_aff", [128, 4], f32, isOutput=False)
    bz_in = nc.declare_dram_parameter("bz", [128, 1], f32, isOutput=False)
    brf_in = nc.declare_dram_parameter("br_fm", [128, HO], f32, isOutput=False)
    btoep_in = nc.declare_dram_parameter("bias_toep", [128, 128], f32, isOutput=False)
    ns_in = nc.declare_dram_parameter("ns", [128, 1], f32, isOutput=False)
    if not zero_bias:
        bv_in = nc.declare_dram_parameter("bv_rep", [128, H], f32, isOutput=False)
        bu_in = nc.declare_dram_parameter("bu_rep", [128, D], f32, isOutput=False)
        bhx_in = nc.declare_dram_parameter("bhx_rep", [128, D], f32, isOutput=False)

    out_p = nc.declare_dram_parameter("out", [L, BC, D], f32, isOutput=True)
    attn_p = nc.declare_dram_parameter("attn", [BC, NCH, CHUNK, CHUNK], f32, isOutput=True)

    # fp16 DRAM scratch, feature-major: (outer, inner128, b, t)
    mx_s = nc.dram_tensor("mx16_s", [DO, 128, BC, L], f16)
    r_s = nc.dram_tensor("r16_s", [HO, 128, BC, L], f16)

    # reordered Wmx columns (host permutes): [u(512) hx(512) r(1024) z(128)]
    CU, CHX, CR, CZ = 0, D, 2 * D, 2 * D + H

    with tile.TileContext(nc) as tc:
        import contextlib
        ctx = contextlib.ExitStack()
        const = ctx.enter_context(tc.tile_pool(name="const", bufs=1))

        Wv_sb = const.tile([128, DO, H], f16)
        nc.sync.dma_start(Wv_sb[:], Wv_in.ap().rearrange("(o p) h -> p o h", p=128))
        Wmx_sb = const.tile([128, DO, 2 * D + H + Z], f16)
        nc.sync.dma_start(Wmx_sb[:], Wmx_in.ap().rearrange("(o p) h -> p o h", p=128))
        Wh_sb = const.tile([128, HO, D], f16)
        nc.sync.dma_start(Wh_sb[:], Wh_in.ap().rearrange("(o p) h -> p o h", p=128))
        emaq_sb = const.tile([128, DO, 4], f32)
        nc.sync.dma_start(emaq_sb[:], emaq_in.ap().rearrange("o p j -> p o j"))
        emaw_sb = const.tile([128, DO, 4], f32)
        nc.sync.dma_start(emaw_sb[:], emaw_in.ap().rearrange("o p j -> p o j"))
        omega_sb = const.tile([128, DO, 1], f32)
        nc.sync.dma_start(omega_sb[:], omega_in.ap().rearrange("o p j -> p o j"))
        qkaff_sb = const.tile([128, 4], f32)
        nc.sync.dma_start(qkaff_sb[:], qkaff_in.ap())
        bz_sb = const.tile([128, 1], f32)
        nc.sync.dma_start(bz_sb[:], bz_in.ap())
        brf_sb = const.tile([128, HO], f32)
        nc.sync.dma_start(brf_sb[:], brf_in.ap())
        btoep_sb = const.tile([128, 128], f32)
        nc.sync.dma_start(btoep_sb[:], btoep_in.ap())
        ns_sb = const.tile([128, 1], f32)
        nc.sync.dma_start(ns_sb[:], ns_in.ap())
        if not zero_bias:
            bv_sb = const.tile([128, H], f32)
            nc.sync.dma_start(bv_sb[:], bv_in.ap())
            bu_sb = const.tile([128, D], f32)
            nc.sync.dma_start(bu_sb[:], bu_in.ap())
            bhx_sb = const.tile([128, D], f32)
            nc.sync.dma_start(bhx_sb[:], bhx_in.ap())
        ident = const.tile([128, 128], f32)
        make_identity(nc, ident[:])
        z_sb = const.tile([128, BC, L], f16)

        # ================= Phase A: EMA scans =================
        with tc.tile_pool(name="pa_big", bufs=2) as big, \
             tc.tile_pool(name="pa_h", bufs=2) as hpool:
            for b in range(BC):
                for do in range(DO):
                    xT_sl = big.tile([128, L], f16, tag="xT_sl")
                    nc.sync.dma_start(xT_sl[:], xT_in.ap()[b, do])

                    h4 = hpool.tile([128, 4, L], f16, tag="h4")
                    for j in range(4):
                        nc.scalar.activation(h4[:, j], xT_sl[:], AF.Copy,
                                             scale=emaw_sb[:, do, j:j + 1])
                        qb = emaq_sb[:, do, j:j + 1].to_broadcast((128, L))
                        if j < 2:
                            nc.vector.tensor_tensor_scan(h4[:, j], qb, h4[:, j],
                                                         0.0, OP.mult, OP.add)
                        else:
                            nc.vector.tensor_tensor_scan(h4[:, j, ::-1], qb, h4[:, j, ::-1],
                                                         0.0, OP.mult, OP.add)
                    ox = big.tile([128, L], f16, tag="ox")
                    nc.scalar.activation(ox[:], xT_sl[:], AF.Copy,
                                         scale=omega_sb[:, do, 0:1])
                    nc.vector.tensor_tensor(h4[:, 0], h4[:, 0], h4[:, 1], OP.add)
                    nc.gpsimd.tensor_tensor(h4[:, 2], h4[:, 2], h4[:, 3], OP.add)
                    nc.vector.tensor_tensor(h4[:, 0], h4[:, 0], h4[:, 2], OP.add)
                    nc.gpsimd.tensor_tensor(h4[:, 0], h4[:, 0], ox[:], OP.add)
                    mx_sl = big.tile([128, L], f16, tag="mx_sl")
                    nc.scalar.activation(mx_sl[:], h4[:, 0], AF.Silu)
                    nc.sync.dma_start(mx_s.ap()[do, :, b, :], mx_sl[:])

        # ================= Phase B: z and r (feature-major, batched) ==========
        with tc.tile_pool(name="pb", bufs=3) as pb, \
             tc.tile_pool(name="pb_ps", bufs=4, space="PSUM") as psb:
            for b in range(BC):
                for ts in range(L // 512):
                    tsl = slice(ts * 512, (ts + 1) * 512)
                    mxz = pb.tile([128, DO, 512], f16, tag="mxz")
                    nc.sync.dma_start(mxz[:], mx_s.ap()[:, :, b, tsl]
                                      .rearrange("o p t -> p o t"))
                    psz = psb.tile([128, 512], f32, tag="mm512")
                    for do in range(DO):
                        nc.tensor.matmul(psz[:], Wmx_sb[:, do, CZ:CZ + Z], mxz[:, do],
                                         start=(do == 0), stop=(do == DO - 1))
                    nc.scalar.activation(z_sb[:, b, tsl], psz[:],
                                         AF.Silu, bias=bz_sb[:, 0:1])
                    for hh in range(HO):
                        psr = psb.tile([128, 512], f32, tag="mm512")
                        for do in range(DO):
                            nc.tensor.matmul(psr[:],
                                             Wmx_sb[:, do, CR + hh * 128:CR + (hh + 1) * 128],
                                             mxz[:, do],
                                             start=(do == 0), stop=(do == DO - 1))
                        rsl = pb.tile([128, 512], f16, tag="rsl")
                        nc.scalar.activation(rsl[:], psr[:], AF.Silu,
                                             bias=brf_sb[:, hh:hh + 1])
                        nc.sync.dma_start(r_s.ap()[hh, :, b, tsl], rsl[:])

        # ================= Phase C: per-chunk stream =================
        ck = ctx.enter_context(tc.tile_pool(name="ck", bufs=3))
        ps512 = ctx.enter_context(tc.tile_pool(name="ps512", bufs=4, space="PSUM"))
        ps128 = ctx.enter_context(tc.tile_pool(name="ps128", bufs=4, space="PSUM"))
        for tidx in range(BC * NCH):
            b, c = tidx // NCH, tidx % NCH
            tok = slice(c * 128, (c + 1) * 128)

            xT_c = ck.tile([128, DO, 128], f16, tag="xT_c")
            nc.sync.dma_start(xT_c[:], xT_in.ap()[b, :, :, tok]
                              .rearrange("o p t -> p o t"))
            mx_c = ck.tile([128, DO, 128], f16, tag="mx_c")
            nc.sync.dma_start(mx_c[:], mx_s.ap()[:, :, b, tok]
                              .rearrange("o p t -> p o t"))
            r_c = ck.tile([128, HO, 128], f16, tag="r_c")
            nc.sync.dma_start(r_c[:], r_s.ap()[:, :, b, tok]
                              .rearrange("o p t -> p o t"))

            # ---- v = silu(x @ Wv [+ bv]) (token-major fp16) ----
            v_c = ck.tile([128, H], f16, tag="v_c")
            for half in range(2):
                hsl = slice(half * 512, (half + 1) * 512)
                psv = ps512.tile([128, 512], f32, tag="mm512")
                for do in range(DO):
                    nc.tensor.matmul(psv[:], xT_c[:, do], Wv_sb[:, do, hsl],
                                     start=(do == 0), stop=(do == DO - 1))
                if zero_bias:
                    nc.scalar.activation(v_c[:, hsl], psv[:], AF.Silu)
                else:
                    vt = ck.tile([128, 512], f32, tag="tmp512a")
                    nc.vector.tensor_tensor(vt[:], psv[:], bv_sb[:, hsl], OP.add)
                    nc.scalar.activation(v_c[:, hsl], vt[:], AF.Silu)

            # ---- u = sigmoid(...) via exp; hx ----
            psu = ps512.tile([128, 512], f32, tag="mm512")
            for do in range(DO):
                nc.tensor.matmul(psu[:], mx_c[:, do], Wmx_sb[:, do, CU:CU + D],
                                 start=(do == 0), stop=(do == DO - 1))
            eu = ck.tile([128, D], f32, tag="eu")
            if zero_bias:
                nc.scalar.activation(eu[:], psu[:], AF.Exp, scale=-1.0)
            else:
                ut = ck.tile([128, 512], f32, tag="tmp512a")
                nc.vector.tensor_tensor(ut[:], psu[:], bu_sb[:], OP.add)
                nc.scalar.activation(eu[:], ut[:], AF.Exp, scale=-1.0)
            nc.vector.tensor_scalar(eu[:], eu[:], 1.0, None, OP.add)
            u_c = ck.tile([128, D], f32, tag="u_c")
            nc.vector.reciprocal(u_c[:], eu[:])

            pshx = ps512.tile([128, 512], f32, tag="mm512")
            for do in range(DO):
                nc.tensor.matmul(pshx[:], mx_c[:, do], Wmx_sb[:, do, CHX:CHX + D],
                                 start=(do == 0), stop=(do == DO - 1))
            hx_c = ck.tile([128, D], f32, tag="hx_c")
            if zero_bias:
                nc.scalar.activation(hx_c[:], pshx[:], AF.Copy)
            else:
                nc.vector.tensor_tensor(hx_c[:], pshx[:], bhx_sb[:], OP.add)

            # ---- attention ----
            q_c = ck.tile([128, 128], f16, tag="q_c")
            nc.vector.tensor_scalar(q_c[:], z_sb[:, b, tok], qkaff_sb[:, 0:1],
                                    qkaff_sb[:, 1:2], OP.mult, OP.add)
            k_c = ck.tile([128, 128], f16, tag="k_c")
            nc.vector.tensor_scalar(k_c[:], z_sb[:, b, tok], qkaff_sb[:, 2:3],
                                    qkaff_sb[:, 3:4], OP.mult, OP.add)
            pss = ps128.tile([128, 128], f32, tag="mm128")
            nc.tensor.matmul(pss[:], q_c[:], k_c[:], start=True, stop=True)
            S_sb = ck.tile([128, 128], f32, tag="S_sb")
            nc.vector.tensor_tensor(S_sb[:], pss[:], btoep_sb[:], OP.add)
            mneg = ck.tile([128, 1], f32, tag="mneg")
            nc.vector.tensor_reduce(mneg[:], S_sb[:], AX.X, OP.max, negate=True)
            E_sb = ck.tile([128, 128], f32, tag="E_sb")
            esum = ck.tile([128, 1], f32, tag="esum")
            nc.scalar.activation(E_sb[:], S_sb[:], AF.Exp, bias=mneg[:, 0:1],
                                 accum_out=esum[:, 0:1])
            rs = ck.tile([128, 1], f32, tag="rs")
            nc.vector.reciprocal(rs[:], esum[:])
            A_sb = ck.tile([128, 128], f32, tag="A_sb")
            nc.vector.tensor_scalar(A_sb[:], E_sb[:], rs[:, 0:1], None, OP.mult)
            nc.sync.dma_start(attn_p.ap()[b, c], A_sb[:])

            psat = ps128.tile([128, 128], f32, tag="mm128")
            nc.tensor.transpose(psat[:], A_sb[:], ident[:])
            at_c = ck.tile([128, 128], f16, tag="at_c")
            nc.scalar.activation(at_c[:], psat[:], AF.Copy)

            # ---- h_attn (feature-major) = (v_c slice).T @ A^T ; rh = h_attn*r ----
            rh = ck.tile([128, HO, 128], f16, tag="rh")
            for hh in range(HO):
                psf = ps128.tile([128, 128], f32, tag="mm128")
                nc.tensor.matmul(psf[:], v_c[:, hh * 128:(hh + 1) * 128], at_c[:],
                                 start=True, stop=True)
                nc.vector.tensor_tensor(rh[:, hh], psf[:], r_c[:, hh], OP.mult)

            # ---- h = silu(hx + rh @ Wh) ----
            pso = ps512.tile([128, 512], f32, tag="mm512")
            for hh in range(HO):
                nc.tensor.matmul(pso[:], rh[:, hh], Wh_sb[:, hh, :],
                                 start=(hh == 0), stop=(hh == HO - 1))
            hpre = ck.tile([128, D], f32, tag="hpre")
            nc.vector.tensor_tensor(hpre[:], pso[:], hx_c[:], OP.add)
            h_c = ck.tile([128, D], f32, tag="h_c")
            nc.scalar.activation(h_c[:], hpre[:], AF.Silu)

            # ---- gated residual + ScaleNorm ----
            x_c = ck.tile([128, D], f32, tag="x_c")
            nc.sync.dma_start(x_c[:], x_in.ap()[tok, b, :])
            d_c = ck.tile([128, D], f32, tag="d_c")
            nc.vector.tensor_tensor(d_c[:], h_c[:], x_c[:], OP.subtract)
            g_c = ck.tile([128, D], f32, tag="g_c")
            nc.gpsimd.tensor_tensor(g_c[:], u_c[:], d_c[:], OP.mult)
            o_c = ck.tile([128, D], f32, tag="o_c")
            nc.gpsimd.tensor_tensor(o_c[:], x_c[:], g_c[:], OP.add)

            sq = ck.tile([128, D], f32, tag="sq")
            nc.gpsimd.tensor_tensor(sq[:], o_c[:], o_c[:], OP.mult)
            ssq = ck.tile([128, 1], f32, tag="ssq")
            nc.vector.tensor_reduce(ssq[:], sq[:], AX.X, OP.add)
            ms = ck.tile([128, 1], f32, tag="ms")
            nc.vector.tensor_scalar(ms[:], ssq[:], 1.0 / D, EPS, OP.mult, OP.add)
            rsq = ck.tile([128, 1], f32, tag="rsq")
            nc.scalar.activation(rsq[:], ms[:], AF.Abs_reciprocal_sqrt)
            scl = ck.tile([128, 1], f32, tag="scl")
            nc.vector.tensor_tensor(scl[:], rsq[:], ns_sb[:], OP.mult)
            outt = ck.tile([128, D], f32, tag="outt")
            nc.gpsimd.tensor_scalar(outt[:], o_c[:], scl[:, 0:1], None, OP.mult)
            nc.sync.dma_start(out_p.ap()[tok, b, :], outt[:])

        ctx.close()

    nc.finalize()
    _PROG_CACHE[key] = nc
    return nc


def _sigmoid(x):
    return 1.0 / (1.0 + np.exp(-x))


def kernel(**inputs):
    from concourse.bass_utils import run_bass_kernel_spmd

    x = np.asarray(inputs["x"], dtype=np.float32)
    Wv = np.asarray(inputs["Wv"], dtype=np.float32)
    bv = np.asarray(inputs["bv"], dtype=np.float32)
    Wmx = np.asarray(inputs["Wmx"], dtype=np.float32)
    bmx = np.asarray(inputs["bmx"], dtype=np.float32)
    Wh = np.asarray(inputs["Wh"], dtype=np.float32)
    bh = np.asarray(inputs["bh"], dtype=np.float32)
    qk_gamma = np.asarray(inputs["qk_gamma"], dtype=np.float64)
    qk_beta = np.asarray(inputs["qk_beta"], dtype=np.float64)
    rel = np.asarray(inputs["rel_pos_bias"], dtype=np.float32)
    ns = np.float32(np.asarray(inputs["norm_scalar"], dtype=np.float32))

    delta = np.asarray(inputs["ema_delta"], dtype=np.float64)[:, :, 0]
    alpha = np.asarray(inputs["ema_alpha"], dtype=np.float64)[:, :, 0]
    beta = np.asarray(inputs["ema_beta"], dtype=np.float64)[:, :, 0]
    gamma = np.asarray(inputs["ema_gamma"], dtype=np.float64)
    omega = np.asarray(inputs["ema_omega"], dtype=np.float32)

    p = _sigmoid(delta)
    q = 1.0 - p * _sigmoid(alpha)
    w = p * beta * gamma / np.sqrt(NDIM)

    qc, qa = q[:D], q[D:]
    wc, wa = w[:D], w[D:]
    ema_q = np.concatenate([qc, qa], axis=1).reshape(DO, 128, 4).astype(np.float32)
    ema_w = np.concatenate([wc, wa], axis=1).reshape(DO, 128, 4).astype(np.float32)
    omega_a = omega.reshape(DO, 128, 1).astype(np.float32)

    # Wmx column reorder: [u(D) hx(D) r(H) z(Z)] from original [u, z, r, hx]
    perm = np.concatenate([
        np.arange(0, D),
        np.arange(D + Z + H, 2 * D + Z + H),
        np.arange(D + Z, D + Z + H),
        np.arange(D, D + Z),
    ])
    Wmx_p = np.ascontiguousarray(Wmx[:, perm])
    bmx_p = bmx[perm]

    zero_bias = bool(np.all(bv == 0) and np.all(bmx_p[:2 * D] == 0)
                     and np.all(bh == 0))
    nc = _build_program(zero_bias)

    bz = np.ascontiguousarray(bmx_p[2 * D + H:, None])
    br_fm = np.ascontiguousarray(
        bmx_p[2 * D:2 * D + H].reshape(HO, 128).T)          # (128, HO)

    s = Z ** -0.5
    qk_aff = np.stack([qk_gamma[0] * s, qk_beta[0] * s,
                       qk_gamma[1], qk_beta[1]], axis=1).astype(np.float32)

    idx = (MAXPOS - 1) + np.arange(CHUNK)[None, :] - np.arange(CHUNK)[:, None]
    bias_toep = np.ascontiguousarray(rel[idx]).astype(np.float32)
    ns_rep = np.full((128, 1), ns, dtype=np.float32)

    shared = dict(Wv16=Wv.astype(np.float16),
                  Wmx16=Wmx_p.astype(np.float16),
                  Wh16=Wh.astype(np.float16),
                  ema_q=ema_q, ema_w=ema_w, omega=omega_a, qk_aff=qk_aff,
                  bz=bz, br_fm=br_fm, bias_toep=bias_toep, ns=ns_rep)
    if not zero_bias:
        ones = np.ones((128, 1), dtype=np.float32)
        shared["bv_rep"] = np.ascontiguousarray(ones * bv[None, :])
        shared["bu_rep"] = np.ascontiguousarray(ones * bmx_p[None, :D])
        shared["bhx_rep"] = np.ascontiguousarray(ones * (bmx_p[D:2 * D] + bh)[None, :])

    in_maps = []
    for k in range(N_CORES):
        xs = np.ascontiguousarray(x[:, k * BC:(k + 1) * BC, :])
        # (L, BC, D) -> (BC, DO, 128, L) fp16 pre-transposed
        xT16 = np.ascontiguousarray(
            xs.transpose(1, 2, 0).reshape(BC, DO, 128, L)).astype(np.float16)
        m = dict(shared)
        m["x"] = xs
        m["xT16"] = xT16
        in_maps.append(m)

    res = run_bass_kernel_spmd(nc, in_maps, core_ids=list(range(N_CORES)))

    out = np.empty((L, B, D), dtype=np.float32)
    attn = np.empty((B, NCH, CHUNK, CHUNK), dtype=np.float32)
    for k in range(N_CORES):
        out[:, k * BC:(k + 1) * BC, :] = res.results[k]["out"]
        attn[k * BC:(k + 1) * BC] = res.results[k]["attn"]
    return out, attn
